# revision 16
# baseline (speedup 1.0000x reference)
"""Self-contained Trainium2 Bass kernel for batched single-head attention.

Problem (hardcoded shapes):
  x [4, 2048, 1024] f32; Wq/Wk/Wv [64, 1024]; bq/bk/bv [64]
  out[b] = softmax((x Wq^T + bq)(x Wk^T + bk)^T / sqrt(64)) (x Wv^T + bv)

Sharding: 8 cores = 4 batches x 2 query-halves. Each core gets the full
x[b]^T (keys/values need the whole sequence) with columns rotated so its
1024 queries are always columns 0-1023 (softmax is key-permutation
invariant, so rotating the key order leaves the output unchanged and lets
all cores run one SPMD program).

Bias algebra: the key bias bk contributes bk.q — a per-query constant —
to every score of that query, which cancels exactly in softmax, so bk is
dropped. The value bias bv contributes P @ (1 bv) / l = bv, so it is
added on the host after the gather. Only bq (scaled) is applied on
device.

Per-core device program (matmul operands bf16, PSUM f32):
  1. DMA x^T [1024, 2048] (bf16) into SBUF; chunk 0 arrives in h-quarters
     and its KV- and Q-projection matmuls are interleaved per quarter so
     the PE tracks DMA arrival with no stalls. ~4 us of dummy matmuls
     first so the PE HAM clock gate is at 2.4 GHz when real work starts.
  2. Software-pipelined chunk loop: S^T slices (K^T slice lhsT vs Q^T
     rhs), exp on ScalarE one slice-pair at a time -> bf16 P^T, and the
     KV projection of chunk c+1 emitted BETWEEN S(c) and O(c) so the PE
     has independent work while the exps drain. K^T copy on DVE (gates
     S), V^T copy on GpSimd (off the critical path).
  3. Tail: O' [65, 1024] (row 64 = softmax denominators l) is copied
     PSUM->SBUF per query-half (ScalarE then DVE) and DMA'd out
     unnormalized; the host divides by l, adds bv, and transposes during
     the gather (0.03% of the kernel FLOPs).
"""

import numpy as np

HIDN = 1024
HEAD = 64
BATCH = 4
SEQ = 2048
NCORES = 8
QH = SEQ // 2  # queries per core
CH = 512  # matmul moving-operand chunk (one f32 PSUM bank)
NH = HIDN // 128  # 8 h-slices
NK = SEQ // 128  # 16 key slices
NCH = SEQ // CH  # 4 column chunks of full seq
NQC = QH // CH  # 2 query chunks

_COMPILED = {}


def _split_multi_waits(nc, max_waits=1):
    """This walrus build rejects instructions carrying more than one sem
    wait ("Too many sync wait commands" in setupSyncWait). Hoist excess
    waits onto same-engine NOPs inserted just before the instruction —
    semantically equivalent (all waits still precede the instruction in
    that engine's stream)."""
    import concourse.mybir as mybir

    n = 0
    for f in nc.m.functions:
        for bb in f.blocks:
            new = []
            dirty = False
            for inst in bb.instructions:
                si = inst.sync_info
                if si is not None and len(si.on_wait) > max_waits:
                    waits = list(si.on_wait)
                    for w in waits[:-max_waits]:
                        nop = mybir.InstNoOp(name=f"wsplit-{n}")
                        n += 1
                        nop.engine = inst.engine
                        nop.sync_info = mybir.SyncInfo(on_wait=[w], on_update=[])
                        new.append(nop)
                    inst.sync_info = mybir.SyncInfo(
                        on_wait=waits[-max_waits:], on_update=list(si.on_update)
                    )
                    dirty = True
                new.append(inst)
            if dirty:
                bb.instructions = new


def _build_nc():
    import concourse.bass as bass
    import concourse.mybir as mybir
    from concourse import masks
    from concourse.tile import TileContext

    f32 = mybir.dt.float32
    bf16 = mybir.dt.bfloat16
    Af = mybir.ActivationFunctionType

    nc = bass.Bass()
    xt_d = nc.declare_dram_parameter("xt", [HIDN, SEQ], bf16, isOutput=False)
    # weights pre-shuffled on host to the SBUF layout [128, h, d]
    wq_d = nc.declare_dram_parameter("wq", [128, NH * HEAD], bf16, isOutput=False)
    wkv_d = nc.declare_dram_parameter("wkv", [128, NH * 128], bf16, isOutput=False)
    bq_d = nc.declare_dram_parameter("bq", [HEAD, 1], f32, isOutput=False)
    # unnormalized output: rows 0:64 = O'^T, row 64 = softmax denominators
    od_d = nc.declare_dram_parameter("od", [HEAD + 1, QH], f32, isOutput=True)

    with TileContext(nc) as tc:
        from contextlib import ExitStack

        with ExitStack() as ctx:
            const_pool = ctx.enter_context(tc.tile_pool(name="const", bufs=1))
            big_pool = ctx.enter_context(tc.tile_pool(name="big", bufs=1))
            # 2 proj banks (cycles warm-up, KV(c), Q(qc), pvt tiles),
            # 4 S banks (2 bufs x 2), 2 O banks = exactly 8 PSUM banks
            ps_proj = ctx.enter_context(
                tc.tile_pool(name="ps_proj", bufs=2, space="PSUM")
            )
            ps_s = ctx.enter_context(tc.tile_pool(name="ps_s", bufs=2, space="PSUM"))
            ps_o = ctx.enter_context(tc.tile_pool(name="ps_o", bufs=1, space="PSUM"))

            # ---- resident SBUF tiles ----
            wq_sb = const_pool.tile([128, NH, HEAD], bf16)
            wkv_sb = const_pool.tile([128, NH, 128], bf16)
            bq_sb = const_pool.tile([HEAD, 1], f32)
            warm_sb = const_pool.tile([128, CH], bf16)
            ident = const_pool.tile([128, 64], bf16)  # identity at partitions 64:128
            xt_sb = big_pool.tile([128, NH, SEQ], bf16)
            qt_sb = big_pool.tile([HEAD, QH], bf16)
            kvt_sb = big_pool.tile([128, SEQ], bf16)  # rows 0:64 K^T, 64:128 V^T
            vones = big_pool.tile([128, NK * (HEAD + 1)], bf16)
            pt_sb = big_pool.tile([128, NK, QH], bf16)
            ot_sb = big_pool.tile([HEAD + 1, QH], f32)

            vones_3d = vones[:].rearrange("p (k e) -> p k e", e=HEAD + 1)

            # ---- x^T DMA on the SP HWDGE ring; chunk 0 in h-quarters so
            # the first projection matmuls start sooner. Weights on the
            # ACT ring (rings are FIFO per issuing engine) ----
            xt_d_r = xt_d[:].rearrange("(h p) s -> p h s", p=128)
            for h4 in range(4):
                nc.sync.dma_start(
                    xt_sb[:, 2 * h4 : 2 * h4 + 2, 0:CH],
                    xt_d_r[:, 2 * h4 : 2 * h4 + 2, 0:CH],
                )
            for c in range(1, NCH):
                nc.sync.dma_start(
                    xt_sb[:, :, c * CH : (c + 1) * CH],
                    xt_d_r[:, :, c * CH : (c + 1) * CH],
                )
            nc.scalar.dma_start(wq_sb[:].rearrange("p h d -> p (h d)"), wq_d[:])
            nc.scalar.dma_start(wkv_sb[:].rearrange("p h d -> p (h d)"), wkv_d[:])
            nc.scalar.dma_start(bq_sb[:], bq_d[:])
            # warm-up tile memset first (GpSimd) so the PE warm-up starts
            # as early as possible; identity/ones after
            nc.gpsimd.memset(warm_sb[:], 0.0)
            masks.make_identity(nc, ident[64:128, :])
            nc.vector.memset(vones_3d[:, :, HEAD : HEAD + 1], 1.0)

            # ---- PE warm-up while the first DMAs are in flight (HAM
            # clock gate at 2.4 GHz when the real matmuls start) ----
            pw = ps_proj.tile([128, CH], f32, tag="ps", name="pw")
            for i in range(9):
                nc.tensor.matmul(
                    pw[:],
                    warm_sb[:, 0:128],
                    warm_sb[:],
                    start=(i == 0),
                    stop=(i == 8),
                )

            po = ps_o.tile([HEAD + 1, QH], f32, tag="po", name="po")

            def proj(c, with_q=None, quarters=False):
                """KV projection of chunk c, optionally interleaved h-wise
                with the Q projection of query-chunk `with_q` (two open
                PSUM accumulation groups in different banks)."""
                cs = slice(c * CH, (c + 1) * CH)
                ps_kv = ps_proj.tile([128, CH], f32, tag="ps", name=f"kv{c}")
                ps_q = None
                if with_q is not None:
                    ps_q = ps_proj.tile([128, CH], f32, tag="ps", name=f"q{with_q}")
                    qs = slice(with_q * CH, (with_q + 1) * CH)
                for h in range(NH):
                    nc.tensor.matmul(
                        ps_kv[:],
                        wkv_sb[:, h, :],
                        xt_sb[:, h, cs],
                        start=(h == 0),
                        stop=(h == NH - 1),
                        skip_group_check=True,
                    )
                    if ps_q is not None:
                        nc.tensor.matmul(
                            ps_q[0:64, :],
                            wq_sb[:, h, :],
                            xt_sb[:, h, qs],
                            start=(h == 0),
                            stop=(h == NH - 1),
                            skip_group_check=True,
                        )
                # K^T copy on DVE gates the S matmuls (emitted first);
                # V^T copy off the critical path (ACT for chunk 0 so the
                # PSUM buffer frees in parallel with K, DVE otherwise).
                # bk/bv are not applied on device (see module docstring).
                nc.vector.tensor_copy(kvt_sb[0:64, cs], ps_kv[0:64, :])
                if ps_q is not None:
                    nc.vector.tensor_scalar_add(
                        qt_sb[:, qs], ps_q[0:64, :], bq_sb[:]
                    )
                if c == 0:
                    nc.scalar.activation(
                        kvt_sb[64:128, cs], ps_kv[64:128, :], Af.Copy
                    )
                else:
                    nc.vector.tensor_copy(kvt_sb[64:128, cs], ps_kv[64:128, :])

            def tr_block(c):
                # V^T -> V transposes into the [V | ones] stationary tile;
                # must be emitted before o_block(c) (a vones read emitted
                # before the copy would have nothing to wait on)
                pvt = ps_proj.tile([128, 4 * HEAD], bf16, tag="ps")
                for t in range(4):
                    k = 4 * c + t
                    nc.tensor.transpose(
                        pvt[:, t * HEAD : (t + 1) * HEAD],
                        kvt_sb[64:128, k * 128 : (k + 1) * 128],
                        ident[64:128, :],
                    )
                nc.vector.tensor_copy(
                    vones_3d[:, 4 * c : 4 * c + 4, 0:HEAD],
                    pvt[:].rearrange("p (k e) -> p k e", e=HEAD),
                )

            def s_pair(a):
                # key-slice pair (a, a+1) x both query chunks; slice-major
                # matmul order so each K^T stationary is loaded once
                pss = [
                    ps_s.tile([128, 2, CH], f32, tag="pss", name=f"pss{qc}")
                    for qc in range(NQC)
                ]
                for t in range(2):
                    for qc in range(NQC):
                        nc.tensor.matmul(
                            pss[qc][:, t, :],
                            kvt_sb[0:64, (a + t) * 128 : (a + t + 1) * 128],
                            qt_sb[:, qc * CH : (qc + 1) * CH],
                            start=True,
                            stop=True,
                        )
                for qc in range(NQC):
                    nc.scalar.activation(
                        pt_sb[:, a : a + 2, qc * CH : (qc + 1) * CH],
                        pss[qc][:],
                        Af.Exp,
                    )

            def s_block(c):
                s_pair(4 * c)
                s_pair(4 * c + 2)

            def o_block(c):
                # slice-major so each [V | ones] stationary is loaded once
                for k in range(4 * c, 4 * c + 4):
                    for qc in range(NQC):
                        nc.tensor.matmul(
                            po[:, qc * CH : (qc + 1) * CH],
                            vones_3d[:, k, :],
                            pt_sb[:, k, qc * CH : (qc + 1) * CH],
                            start=(k == 0),
                            stop=(k == NK - 1),
                        )

            # ---- software-pipelined schedule: proj(c+2) sits between
            # S(c) and O(c) so the PE has independent work while the
            # exps of chunk c drain on ScalarE ----
            proj(0, with_q=0)
            proj(1, with_q=1)
            s_block(0)
            tr_block(0)
            proj(2)
            o_block(0)
            s_block(1)
            tr_block(1)
            proj(3)
            o_block(1)
            s_block(2)
            tr_block(2)
            o_block(2)
            s_block(3)
            tr_block(3)
            o_block(3)

            # ---- tail: O' (denominators in row 64) straight out;
            # normalization/bv/transpose happen on the host during gather.
            # PSUM isn't DMA-able, so bounce through SBUF ----
            nc.scalar.activation(ot_sb[:, 0:CH], po[:, 0:CH], Af.Copy)
            nc.sync.dma_start(od_d[:, 0:CH], ot_sb[:, 0:CH])
            nc.vector.tensor_copy(ot_sb[:, CH:QH], po[:, CH:QH])
            nc.sync.dma_start(od_d[:, CH:QH], ot_sb[:, CH:QH])

    _split_multi_waits(nc)
    return nc


def _get_nc():
    if "nc" not in _COMPILED:
        _COMPILED["nc"] = _build_nc()
    return _COMPILED["nc"]


def make_in_maps(x, Wq, bq, Wk, bk, Wv, bv):
    import ml_dtypes

    bf16 = ml_dtypes.bfloat16
    x = np.asarray(x, np.float32)
    scale = np.float32(1.0 / np.sqrt(HEAD))

    xT = np.ascontiguousarray(x.transpose(0, 2, 1))  # [4, 1024, 2048] f32

    def shuffle_w(wt):  # [1024, d] -> SBUF layout [128, 8*d]
        d = wt.shape[1]
        return np.ascontiguousarray(
            wt.reshape(NH, 128, d).transpose(1, 0, 2).reshape(128, NH * d)
        )

    wq = shuffle_w(np.asarray(Wq, np.float32).T * scale).astype(bf16)
    wkv = shuffle_w(
        np.concatenate(
            [np.asarray(Wk, np.float32).T, np.asarray(Wv, np.float32).T], axis=1
        )
    ).astype(bf16)
    bqs = (np.asarray(bq, np.float32) * scale).reshape(HEAD, 1)

    in_maps = []
    for c in range(NCORES):
        b, qh = c // 2, c % 2
        if qh == 0:
            xt_c = xT[b]
        else:
            # rotate so this core's queries are columns 0:1024; key-order
            # permutation does not change softmax attention output
            xt_c = np.concatenate([xT[b][:, QH:], xT[b][:, :QH]], axis=1)
        in_maps.append(
            {
                "xt": np.ascontiguousarray(xt_c).astype(bf16),
                "wq": wq,
                "wkv": wkv,
                "bq": bqs,
            }
        )
    return in_maps


def gather_out(results, bv=None):
    out = np.empty((BATCH, SEQ, HEAD), np.float32)
    bv = np.zeros(HEAD, np.float32) if bv is None else np.asarray(bv, np.float32)
    for c in range(NCORES):
        b, qh = c // 2, c % 2
        od = results[c]["od"]  # [65, 1024]: rows 0:64 O'^T, row 64 = l
        out[b, qh * QH : (qh + 1) * QH, :] = (od[0:HEAD, :] / od[HEAD, :]).T + bv
    return out


def kernel(x, Wq, bq, Wk, bk, Wv, bv):
    nc = _get_nc()
    in_maps = make_in_maps(x, Wq, bq, Wk, bk, Wv, bv)

    from concourse.bass_utils import run_bass_kernel_spmd

    res = run_bass_kernel_spmd(nc, in_maps, list(range(NCORES)))
    return gather_out(res.results, bv)


# revision 18
# speedup vs baseline: 1.0398x; 1.0398x over previous
"""Self-contained Trainium2 Bass kernel for batched single-head attention.

Problem (hardcoded shapes):
  x [4, 2048, 1024] f32; Wq/Wk/Wv [64, 1024]; bq/bk/bv [64]
  out[b] = softmax((x Wq^T + bq)(x Wk^T + bk)^T / sqrt(64)) (x Wv^T + bv)

Sharding: 8 cores = 4 batches x 2 query-halves. Each core gets the full
x[b]^T (keys/values need the whole sequence) with columns rotated so its
1024 queries are always columns 0-1023 (softmax is key-permutation
invariant, so rotating the key order leaves the output unchanged and lets
all cores run one SPMD program).

Bias algebra: the key bias bk contributes bk.q — a per-query constant —
to every score of that query, which cancels exactly in softmax, so bk is
dropped. The value bias bv contributes P @ (1 bv) / l = bv, so it is
added on the host after the gather. Only bq (scaled) is applied on
device.

Per-core device program (matmul operands bf16, PSUM f32):
  1. DMA x^T [1024, 2048] (bf16) into SBUF; chunk 0 arrives in h-quarters
     and its KV- and Q-projection matmuls are interleaved per quarter so
     the PE tracks DMA arrival with no stalls. ~4 us of dummy matmuls
     first so the PE HAM clock gate is at 2.4 GHz when real work starts.
  2. Software-pipelined chunk loop: S^T slices (K^T slice lhsT vs Q^T
     rhs), exp on ScalarE one slice-pair at a time -> bf16 P^T, and the
     KV projection of chunk c+1 emitted BETWEEN S(c) and O(c) so the PE
     has independent work while the exps drain. K^T copy on DVE (gates
     S), V^T copy on GpSimd (off the critical path).
  3. Tail: O' [65, 1024] (row 64 = softmax denominators l) is copied
     PSUM->SBUF per query-half (ScalarE then DVE) and DMA'd out
     unnormalized; the host divides by l, adds bv, and transposes during
     the gather (0.03% of the kernel FLOPs).
"""

import numpy as np

HIDN = 1024
HEAD = 64
BATCH = 4
SEQ = 2048
NCORES = 8
QH = SEQ // 2  # queries per core
CH = 512  # matmul moving-operand chunk (one f32 PSUM bank)
NH = HIDN // 128  # 8 h-slices
NK = SEQ // 128  # 16 key slices
NCH = SEQ // CH  # 4 column chunks of full seq
NQC = QH // CH  # 2 query chunks

_COMPILED = {}


def _split_multi_waits(nc, max_waits=1):
    """This walrus build rejects instructions carrying more than one sem
    wait ("Too many sync wait commands" in setupSyncWait). Hoist excess
    waits onto same-engine NOPs inserted just before the instruction —
    semantically equivalent (all waits still precede the instruction in
    that engine's stream)."""
    import concourse.mybir as mybir

    n = 0
    for f in nc.m.functions:
        for bb in f.blocks:
            new = []
            dirty = False
            for inst in bb.instructions:
                si = inst.sync_info
                if si is not None and len(si.on_wait) > max_waits:
                    waits = list(si.on_wait)
                    for w in waits[:-max_waits]:
                        nop = mybir.InstNoOp(name=f"wsplit-{n}")
                        n += 1
                        nop.engine = inst.engine
                        nop.sync_info = mybir.SyncInfo(on_wait=[w], on_update=[])
                        new.append(nop)
                    inst.sync_info = mybir.SyncInfo(
                        on_wait=waits[-max_waits:], on_update=list(si.on_update)
                    )
                    dirty = True
                new.append(inst)
            if dirty:
                bb.instructions = new


def _build_nc():
    import concourse.bass as bass
    import concourse.mybir as mybir
    from concourse import masks
    from concourse.tile import TileContext

    f32 = mybir.dt.float32
    bf16 = mybir.dt.bfloat16
    Af = mybir.ActivationFunctionType

    nc = bass.Bass()
    xt_d = nc.declare_dram_parameter("xt", [HIDN, SEQ], bf16, isOutput=False)
    # weights pre-shuffled on host to the SBUF layout [128, h, d]
    wq_d = nc.declare_dram_parameter("wq", [128, NH * HEAD], bf16, isOutput=False)
    wkv_d = nc.declare_dram_parameter("wkv", [128, NH * 128], bf16, isOutput=False)
    bq_d = nc.declare_dram_parameter("bq", [HEAD, 1], f32, isOutput=False)
    # unnormalized output: rows 0:64 = O'^T, row 64 = softmax denominators
    od_d = nc.declare_dram_parameter("od", [HEAD + 1, QH], f32, isOutput=True)

    with TileContext(nc) as tc:
        from contextlib import ExitStack

        with ExitStack() as ctx:
            const_pool = ctx.enter_context(tc.tile_pool(name="const", bufs=1))
            big_pool = ctx.enter_context(tc.tile_pool(name="big", bufs=1))
            # 2 proj banks (cycles warm-up, KV(c), Q(qc), pvt tiles),
            # 4 S banks (2 bufs x 2), 2 O banks = exactly 8 PSUM banks
            ps_proj = ctx.enter_context(
                tc.tile_pool(name="ps_proj", bufs=2, space="PSUM")
            )
            ps_s = ctx.enter_context(tc.tile_pool(name="ps_s", bufs=2, space="PSUM"))
            ps_o = ctx.enter_context(tc.tile_pool(name="ps_o", bufs=1, space="PSUM"))

            # ---- resident SBUF tiles ----
            wq_sb = const_pool.tile([128, NH, HEAD], bf16)
            wkv_sb = const_pool.tile([128, NH, 128], bf16)
            bq_sb = const_pool.tile([HEAD, 1], f32)
            warm_sb = const_pool.tile([128, CH], bf16)
            ident = const_pool.tile([128, 64], bf16)  # identity at partitions 64:128
            xt_sb = big_pool.tile([128, NH, SEQ], bf16)
            qt_sb = big_pool.tile([HEAD, QH], bf16)
            kvt_sb = big_pool.tile([128, SEQ], bf16)  # rows 0:64 K^T, 64:128 V^T
            vones = big_pool.tile([128, NK * (HEAD + 1)], bf16)
            pt_sb = big_pool.tile([128, NK, QH], bf16)
            ot_sb = big_pool.tile([HEAD + 1, QH], f32)

            vones_3d = vones[:].rearrange("p (k e) -> p k e", e=HEAD + 1)

            # ---- x^T DMA on the SP HWDGE ring; chunk 0 in h-quarters so
            # the first projection matmuls start sooner. Weights on the
            # ACT ring (rings are FIFO per issuing engine) ----
            xt_d_r = xt_d[:].rearrange("(h p) s -> p h s", p=128)
            for h4 in range(4):
                nc.sync.dma_start(
                    xt_sb[:, 2 * h4 : 2 * h4 + 2, 0:CH],
                    xt_d_r[:, 2 * h4 : 2 * h4 + 2, 0:CH],
                )
            for c in range(1, NCH):
                nc.sync.dma_start(
                    xt_sb[:, :, c * CH : (c + 1) * CH],
                    xt_d_r[:, :, c * CH : (c + 1) * CH],
                )
            nc.scalar.dma_start(wq_sb[:].rearrange("p h d -> p (h d)"), wq_d[:])
            nc.scalar.dma_start(wkv_sb[:].rearrange("p h d -> p (h d)"), wkv_d[:])
            nc.scalar.dma_start(bq_sb[:], bq_d[:])
            # warm-up tile memset first (GpSimd) so the PE warm-up starts
            # as early as possible; identity/ones after
            nc.gpsimd.memset(warm_sb[:], 0.0)
            masks.make_identity(nc, ident[64:128, :])
            nc.vector.memset(vones_3d[:, :, HEAD : HEAD + 1], 1.0)

            # ---- PE warm-up while the first DMAs are in flight (HAM
            # clock gate at 2.4 GHz when the real matmuls start) ----
            pw = ps_proj.tile([128, CH], f32, tag="ps", name="pw")
            for i in range(9):
                nc.tensor.matmul(
                    pw[:],
                    warm_sb[:, 0:128],
                    warm_sb[:],
                    start=(i == 0),
                    stop=(i == 8),
                )

            po = ps_o.tile([HEAD + 1, QH], f32, tag="po", name="po")

            def proj(c, with_q=None, quarters=False):
                """KV projection of chunk c, optionally interleaved h-wise
                with the Q projection of query-chunk `with_q` (two open
                PSUM accumulation groups in different banks)."""
                cs = slice(c * CH, (c + 1) * CH)
                ps_kv = ps_proj.tile([128, CH], f32, tag="ps", name=f"kv{c}")
                ps_q = None
                if with_q is not None:
                    ps_q = ps_proj.tile([128, CH], f32, tag="ps", name=f"q{with_q}")
                    qs = slice(with_q * CH, (with_q + 1) * CH)
                for h in range(NH):
                    nc.tensor.matmul(
                        ps_kv[:],
                        wkv_sb[:, h, :],
                        xt_sb[:, h, cs],
                        start=(h == 0),
                        stop=(h == NH - 1),
                        skip_group_check=True,
                    )
                    if ps_q is not None:
                        nc.tensor.matmul(
                            ps_q[0:64, :],
                            wq_sb[:, h, :],
                            xt_sb[:, h, qs],
                            start=(h == 0),
                            stop=(h == NH - 1),
                            skip_group_check=True,
                        )
                # K^T copy on DVE gates the S matmuls (emitted first);
                # V^T copy off the critical path (ACT for chunk 0 so the
                # PSUM buffer frees in parallel with K, DVE otherwise).
                # bk/bv are not applied on device (see module docstring).
                nc.vector.tensor_copy(kvt_sb[0:64, cs], ps_kv[0:64, :])
                if ps_q is not None:
                    nc.vector.tensor_scalar_add(
                        qt_sb[:, qs], ps_q[0:64, :], bq_sb[:]
                    )
                if c == 0:
                    nc.scalar.activation(
                        kvt_sb[64:128, cs], ps_kv[64:128, :], Af.Copy
                    )
                else:
                    nc.vector.tensor_copy(kvt_sb[64:128, cs], ps_kv[64:128, :])

            def tr_block(c):
                # V^T -> V transposes into the [V | ones] stationary tile;
                # must be emitted before o_block(c) (a vones read emitted
                # before the copy would have nothing to wait on)
                pvt = ps_proj.tile([128, 4 * HEAD], bf16, tag="ps")
                for t in range(4):
                    k = 4 * c + t
                    nc.tensor.transpose(
                        pvt[:, t * HEAD : (t + 1) * HEAD],
                        kvt_sb[64:128, k * 128 : (k + 1) * 128],
                        ident[64:128, :],
                    )
                nc.vector.tensor_copy(
                    vones_3d[:, 4 * c : 4 * c + 4, 0:HEAD],
                    pvt[:].rearrange("p (k e) -> p k e", e=HEAD),
                )

            def s_pair(a):
                # key-slice pair (a, a+1) x both query chunks; palindrome
                # slice order (a,a+1 | a+1,a) shares the middle stationary
                # load while exp(qc0) still fires after only two matmuls
                pss = [
                    ps_s.tile([128, 2, CH], f32, tag="pss", name=f"pss{qc}")
                    for qc in range(NQC)
                ]
                for qc in range(NQC):
                    for t in ((0, 1) if qc == 0 else (1, 0)):
                        nc.tensor.matmul(
                            pss[qc][:, t, :],
                            kvt_sb[0:64, (a + t) * 128 : (a + t + 1) * 128],
                            qt_sb[:, qc * CH : (qc + 1) * CH],
                            start=True,
                            stop=True,
                        )
                    nc.scalar.activation(
                        pt_sb[:, a : a + 2, qc * CH : (qc + 1) * CH],
                        pss[qc][:],
                        Af.Exp,
                    )

            def s_block(c):
                s_pair(4 * c)
                s_pair(4 * c + 2)

            def o_block(c):
                for j in range(2):
                    for qc in range(NQC):
                        for t in range(2):
                            k = 4 * c + 2 * j + t
                            nc.tensor.matmul(
                                po[:, qc * CH : (qc + 1) * CH],
                                vones_3d[:, k, :],
                                pt_sb[:, k, qc * CH : (qc + 1) * CH],
                                start=(k == 0),
                                stop=(k == NK - 1),
                            )

            # ---- software-pipelined schedule: proj(c+2) sits between
            # S(c) and O(c) so the PE has independent work while the
            # exps of chunk c drain on ScalarE ----
            proj(0, with_q=0)
            proj(1, with_q=1)
            s_block(0)
            tr_block(0)
            proj(2)
            o_block(0)
            s_block(1)
            tr_block(1)
            proj(3)
            o_block(1)
            s_block(2)
            tr_block(2)
            o_block(2)
            s_block(3)
            tr_block(3)
            o_block(3)

            # ---- tail: O' (denominators in row 64) straight out;
            # normalization/bv/transpose happen on the host during gather.
            # PSUM isn't DMA-able, so bounce through SBUF ----
            nc.scalar.activation(ot_sb[:, 0:CH], po[:, 0:CH], Af.Copy)
            nc.sync.dma_start(od_d[:, 0:CH], ot_sb[:, 0:CH])
            nc.vector.tensor_copy(ot_sb[:, CH:QH], po[:, CH:QH])
            nc.sync.dma_start(od_d[:, CH:QH], ot_sb[:, CH:QH])

    _split_multi_waits(nc)
    return nc


def _get_nc():
    if "nc" not in _COMPILED:
        _COMPILED["nc"] = _build_nc()
    return _COMPILED["nc"]


def make_in_maps(x, Wq, bq, Wk, bk, Wv, bv):
    import ml_dtypes

    bf16 = ml_dtypes.bfloat16
    x = np.asarray(x, np.float32)
    scale = np.float32(1.0 / np.sqrt(HEAD))

    xT = np.ascontiguousarray(x.transpose(0, 2, 1))  # [4, 1024, 2048] f32

    def shuffle_w(wt):  # [1024, d] -> SBUF layout [128, 8*d]
        d = wt.shape[1]
        return np.ascontiguousarray(
            wt.reshape(NH, 128, d).transpose(1, 0, 2).reshape(128, NH * d)
        )

    wq = shuffle_w(np.asarray(Wq, np.float32).T * scale).astype(bf16)
    wkv = shuffle_w(
        np.concatenate(
            [np.asarray(Wk, np.float32).T, np.asarray(Wv, np.float32).T], axis=1
        )
    ).astype(bf16)
    bqs = (np.asarray(bq, np.float32) * scale).reshape(HEAD, 1)

    in_maps = []
    for c in range(NCORES):
        b, qh = c // 2, c % 2
        if qh == 0:
            xt_c = xT[b]
        else:
            # rotate so this core's queries are columns 0:1024; key-order
            # permutation does not change softmax attention output
            xt_c = np.concatenate([xT[b][:, QH:], xT[b][:, :QH]], axis=1)
        in_maps.append(
            {
                "xt": np.ascontiguousarray(xt_c).astype(bf16),
                "wq": wq,
                "wkv": wkv,
                "bq": bqs,
            }
        )
    return in_maps


def gather_out(results, bv=None):
    out = np.empty((BATCH, SEQ, HEAD), np.float32)
    bv = np.zeros(HEAD, np.float32) if bv is None else np.asarray(bv, np.float32)
    for c in range(NCORES):
        b, qh = c // 2, c % 2
        od = results[c]["od"]  # [65, 1024]: rows 0:64 O'^T, row 64 = l
        out[b, qh * QH : (qh + 1) * QH, :] = (od[0:HEAD, :] / od[HEAD, :]).T + bv
    return out


def kernel(x, Wq, bq, Wk, bk, Wv, bv):
    nc = _get_nc()
    in_maps = make_in_maps(x, Wq, bq, Wk, bk, Wv, bv)

    from concourse.bass_utils import run_bass_kernel_spmd

    res = run_bass_kernel_spmd(nc, in_maps, list(range(NCORES)))
    return gather_out(res.results, bv)
